# revision 40
# baseline (speedup 1.0000x reference)
"""Trainium2 Bass kernel for nn_DualOutputMoE.

Math: the reference collapses the whole MoE into a single [1,1,H] vector:
    acc = sum_e combine[:,e] @ (gelu(x @ W1[e] + b1[e]) @ W2[e] + b2[e])
    out = acc / total_weight
Since combine is applied *linearly* after the gelu, we contract it with the
gelu activations first:
    u_e  = combine[:,e] @ gelu(x @ W1[e] + b1[e])          # [F]
    acc  = sum_e (u_e @ W2[e] + combine[:,e].sum() * b2[e])
which turns the second [T,F]@[F,H] matmul into an [F]@[F,H] matvec.
Only tokens actually routed to expert e (combine[:,e] != 0) contribute, so we
gather those tokens on the host (top-2 of 16 experts -> ~T/8 tokens/expert)
and the device computes a dense [C,H]@[H,F] per expert with C = padded
capacity.

Sharding: expert-parallel, 2 experts per core across 8 cores (E=16). Each
core returns its partial [1,H] accumulator; the host sums them.

Device kernel (per core, SPMD over 8 cores, bf16 with fp32 PSUM):
  mm1:  psA[tok128, f512] += xgT[h128, tok128].T @ W1[h128, f512]   (8 k-tiles)
  gelu: G = gelu(psA)                     (ACT, psum->sbuf bf16)
  cmm:  psB[1, f512] += c[tok128, 1].T @ G                          (NT tok-tiles)
  u     -> [8, 512] sbuf -> DRAM -> uT [128, F/128] (partition transpose)
  mm2:  psC[1, h512] += uT[f128, 1].T @ W2[f128, h512]   (32 f-tiles, 2 experts)

Schedule: expert 0's mm2 is interleaved into expert 1's phase 1 (PE program
order is emission order), and W2 tiles are prefetched into SBUF during phase 1
so mm2 never waits on DMA.
"""

import sys
import math

if "/opt/trn_rl_repo" not in sys.path:
    sys.path.insert(0, "/opt/trn_rl_repo")

import numpy as np
import ml_dtypes

import concourse.bass as bass
import concourse.tile as tile
from concourse import bacc, mybir
from concourse.bass_utils import run_bass_kernel_spmd

BF16 = ml_dtypes.bfloat16
FP8 = ml_dtypes.float8_e4m3
N_CORES = 8
E = 16
EPC = E // N_CORES  # experts per core
H = 1024
F = 4096
TOP_K = 2
KH = H // 128  # 8 bf16 k-tiles along H
KH2 = H // 256  # 4 fp8-DoubleRow k-tiles along H (256 contraction per tile)
FT = F // 128  # 32 f-tiles along F
FC = F // 512  # 8 f-chunks of 512
HC = H // 512  # 2 h-chunks of 512

USE_FP8 = True  # fp8(e4m3) DoubleRow for mm1 (2x PE throughput vs bf16)
W1_SCALE = 32.0  # pre-scale W1 into fp8's normal range; gelu un-scales

_compiled_cache = {}


def _build(nt: int, has_b1: bool, reps: int = 1):
    """Build + compile the SPMD device program for NT token tiles per expert.

    reps > 1 wraps the whole body in a hardware For_i loop running it that
    many times (used by test.py for wall-clock timing).
    """
    key = (nt, has_b1, reps)
    if key in _compiled_cache:
        return _compiled_cache[key]

    C = nt * 128
    nc = bacc.Bacc("TRN2", target_bir_lowering=False, debug=False)
    f32 = mybir.dt.float32
    bf16 = mybir.dt.bfloat16
    fp8 = mybir.dt.float8e4

    if USE_FP8:
        # mm1 operands pre-interleaved for DoubleRow: h = kt*256 + p*2 + j.
        # w1 is fc-major so each f-chunk is one contiguous DMA with 1KB lines.
        xg_d = nc.dram_tensor(
            "xg", [EPC, KH2, 128, 2, C], fp8, kind="ExternalInput"
        ).ap()
        w1_d = nc.dram_tensor(
            "w1", [EPC, FC, KH2, 128, 2 * 512], fp8, kind="ExternalInput"
        ).ap()
    else:
        xg_d = nc.dram_tensor("xg", [EPC, H, C], bf16, kind="ExternalInput").ap()
        w1_d = nc.dram_tensor("w1", [EPC, H, F], bf16, kind="ExternalInput").ap()
    cw_d = nc.dram_tensor("cw", [EPC, C], bf16, kind="ExternalInput").ap()
    w2_d = nc.dram_tensor("w2", [EPC, F, H], bf16, kind="ExternalInput").ap()
    if has_b1:
        b1_d = nc.dram_tensor("b1", [EPC, F], bf16, kind="ExternalInput").ap()
    acc_d = nc.dram_tensor("acc", [1, H], f32, kind="ExternalOutput").ap()
    u_d = nc.dram_tensor("u_scratch", [EPC, F], f32).ap()

    with tile.TileContext(nc) as tc:
        with (
            tc.tile_pool(name="xg", bufs=1) as xg_pool,
            tc.tile_pool(name="cw", bufs=1) as cw_pool,
            tc.tile_pool(name="w1", bufs=2) as w1_pool,
            tc.tile_pool(name="w2", bufs=20) as w2_pool,
            tc.tile_pool(name="g", bufs=12) as g_pool,
            tc.tile_pool(name="u", bufs=1) as u_pool,
            tc.tile_pool(name="small", bufs=1) as small_pool,
            tc.tile_pool(name="psA", bufs=3, space="PSUM") as psA_pool,
            tc.tile_pool(name="psB", bufs=2, space="PSUM") as psB_pool,
            tc.tile_pool(name="psC", bufs=1, space="PSUM") as psC_pool,
        ):
            xg_sb, cw_sb, b1_sb, u8, uT_f, uT_b = [], [], [], [], [], []
            ones_sb = None
            NKT = KH2 if USE_FP8 else KH  # mm1 k-tiles per (tt, fc)
            for e in range(EPC):
                if USE_FP8:
                    xg_sb.append(
                        xg_pool.tile(
                            [128, KH2, 2, C], fp8, tag=f"xg{e}", name=f"xg{e}"
                        )
                    )
                else:
                    xg_sb.append(
                        xg_pool.tile([128, KH, C], bf16, tag=f"xg{e}", name=f"xg{e}")
                    )
                cw_sb.append(
                    cw_pool.tile([128, nt], bf16, tag=f"cw{e}", name=f"cw{e}")
                )
                u8.append(u_pool.tile([1, F], f32, tag=f"u{e}", name=f"u{e}"))
                uT_f.append(
                    small_pool.tile([128, FT], f32, tag=f"uTf{e}", name=f"uTf{e}")
                )
                uT_b.append(
                    small_pool.tile([128, FT], bf16, tag=f"uTb{e}", name=f"uTb{e}")
                )

            def load_xg(e, kh):
                # per-k-tile loads so the first matmul starts after ~0.3 MB
                if USE_FP8:
                    nc.sync.dma_start(xg_sb[e][:, kh, :, :], xg_d[e, kh])
                else:
                    nc.sync.dma_start(
                        xg_sb[e][:, kh, :], xg_d[e, kh * 128 : (kh + 1) * 128, :]
                    )

            def load_cw(e):
                nc.sync.dma_start(
                    cw_sb[e][:], cw_d[e].rearrange("(t p) -> p t", p=128)
                )

            if has_b1:
                ones_sb = small_pool.tile([1, 128], bf16, tag="ones", name="ones")
                nc.vector.memset(ones_sb[:], 1.0)
                for e in range(EPC):
                    b1_t = small_pool.tile([1, F], bf16, tag=f"b1{e}", name=f"b1{e}")
                    nc.sync.dma_start(b1_t[:], b1_d[e : e + 1, :])
                    b1_sb.append(b1_t)

            def w2_prefetch(e, ft):
                w2_t = w2_pool.tile([128, HC, 512], bf16, name="w2t")
                nc.sync.dma_start(
                    w2_t[:],
                    w2_d[e, ft * 128 : (ft + 1) * 128, :].rearrange(
                        "p (h n) -> p h n", h=HC
                    ),
                )
                return w2_t

            # software-pipelined state (emitted later than produced)
            cmm_q = []  # queued combine-matvecs: (e, fc, tt, g_tile)
            mm2_q = []  # queued mm2 chunk batches: (e, fc, [w2 tiles])
            state = {"mm2_count": 0}
            N_MM2 = EPC * FT * HC

            def pop_cmm():
                if not cmm_q:
                    return
                e, fc, tt, g_t = cmm_q.pop(0)
                nc.tensor.matmul(
                    psB_cur[(e, fc)][:],
                    lhsT=cw_sb[e][:, tt : tt + 1],
                    rhs=g_t[:],
                    start=(tt == 0),
                    stop=(tt == nt - 1),
                )
                if tt == nt - 1:
                    finish_chunk(e, fc)

            def finish_chunk(e, fc):
                # psB -> u8 row -> DRAM -> uT columns -> bf16 cast; then queue
                # this chunk's mm2 work
                psB = psB_cur.pop((e, fc))
                nc.vector.tensor_copy(
                    u8[e][:, fc * 512 : (fc + 1) * 512], psB[:]
                )
                nc.sync.dma_start(
                    u_d[e : e + 1, fc * 512 : (fc + 1) * 512],
                    u8[e][:, fc * 512 : (fc + 1) * 512],
                )
                nc.sync.dma_start(
                    uT_f[e][:, 4 * fc : 4 * fc + 4],
                    u_d[e, fc * 512 : (fc + 1) * 512].rearrange(
                        "(j p) -> p j", p=128
                    ),
                )
                nc.vector.tensor_copy(
                    uT_b[e][:, 4 * fc : 4 * fc + 4],
                    uT_f[e][:, 4 * fc : 4 * fc + 4],
                )
                w2_tiles = [w2_prefetch(e, ft) for ft in range(4 * fc, 4 * fc + 4)]
                mm2_q.append((e, fc, w2_tiles))

            def pop_mm2(min_q=0):
                # min_q>0 keeps batches queued so PE has mm2 work to chew on
                # while the final u-chunks round-trip through DRAM
                if len(mm2_q) <= min_q:
                    return
                e, fc, w2_tiles = mm2_q.pop(0)
                for j, ft in enumerate(range(4 * fc, 4 * fc + 4)):
                    for hc in range(HC):
                        # first/last matmul into EACH psC bank get start/stop
                        nc.tensor.matmul(
                            psC[hc][:],
                            lhsT=uT_b[e][:, ft : ft + 1],
                            rhs=w2_tiles[j][:, hc, :],
                            start=(state["mm2_count"] < HC),
                            stop=(state["mm2_count"] >= N_MM2 - HC),
                        )
                        state["mm2_count"] += 1

            def load_w1(e, fc, split=False):
                fsl = slice(fc * 512, (fc + 1) * 512)
                if USE_FP8:
                    w1_t = w1_pool.tile([128, KH2, 2 * 512], fp8, name="w1t")
                    if split:  # per-k-tile DMAs: first mm waits on 128KB only
                        for kt in range(KH2):
                            nc.sync.dma_start(
                                w1_t[:, kt, :], w1_d[e, fc, kt]
                            )
                    else:
                        nc.sync.dma_start(
                            w1_t[:],
                            w1_d[e, fc].rearrange("k p n -> p k n"),
                        )
                    return w1_t
                w1_t = w1_pool.tile([128, KH, 512], bf16, name="w1t")
                if split:
                    for kh in range(KH):
                        nc.sync.dma_start(
                            w1_t[:, kh, :],
                            w1_d[e, kh * 128 : (kh + 1) * 128, fsl],
                        )
                else:
                    nc.sync.dma_start(
                        w1_t[:],
                        w1_d[e, :, fsl].rearrange("(k p) n -> p k n", p=128),
                    )
                return w1_t

            def mm1_block(e, fc, w1_t=None):
                """mm1s for one f-chunk (w1 DMA + nt token-tile groups); the
                gelu-dependent cmms are emitted one tt-group later via cmm_q."""
                if w1_t is None:
                    w1_t = load_w1(e, fc)
                psB_cur[(e, fc)] = psB_pool.tile([1, 512], f32, name="psB")
                for tt in range(nt):
                    psA = psA_pool.tile([128, 512], f32, name="psA")
                    tsl = slice(tt * 128, (tt + 1) * 128)
                    if USE_FP8:
                        for kt in range(KH2):
                            nc.tensor.matmul(
                                psA[:],
                                lhsT=xg_sb[e][:, kt, :, tsl],
                                rhs=w1_t[:, kt, :].rearrange(
                                    "p (j n) -> p j n", j=2
                                ),
                                start=(kt == 0),
                                stop=(kt == KH2 - 1) and not has_b1,
                                perf_mode=mybir.MatmulPerfMode.DoubleRow,
                            )
                    else:
                        for kh in range(KH):
                            nc.tensor.matmul(
                                psA[:],
                                lhsT=xg_sb[e][:, kh, tsl],
                                rhs=w1_t[:, kh, :],
                                start=(kh == 0),
                                stop=(kh == KH - 1) and not has_b1,
                            )
                    if has_b1:
                        nc.tensor.matmul(
                            psA[:],
                            lhsT=ones_sb[:],
                            rhs=b1_sb[e][:, fc * 512 : (fc + 1) * 512],
                            start=False,
                            stop=True,
                        )
                    g_t = g_pool.tile([128, 512], bf16, name="gt")
                    nc.scalar.activation(
                        g_t[:],
                        psA[:],
                        mybir.ActivationFunctionType.Gelu,
                        scale=(1.0 / W1_SCALE) if USE_FP8 else 1.0,
                    )
                    cmm_q.append((e, fc, tt, g_t))

            def emit_body():
                psC.clear()
                psC.extend(
                    psC_pool.tile([1, 512], f32, tag=f"psC{hc}", name=f"psC{hc}")
                    for hc in range(HC)
                )
                state["mm2_count"] = 0
                # startup loads live INSIDE the body: with reps>1 every
                # iteration re-reads them (same work as a single-shot run),
                # and w1_first's pool slot is rewritten before reuse.
                load_xg(0, 0)
                load_cw(0)
                w1_first = load_w1(0, 0, split=True)
                for kh in range(1, NKT):
                    load_xg(0, kh)
                for e in range(EPC):
                    for fc in range(FC):
                        if e == 0 and fc < NKT:  # spread e1's input loads out
                            load_xg(1, fc)
                            if fc == 0:
                                load_cw(1)
                        mm1_block(e, fc, w1_t=w1_first if (e, fc) == (0, 0) else None)
                        # previous fc's cmms in ONE batch (fewer DR-stream
                        # breaks; their gelus finished during this fc's mm1s)
                        while len(cmm_q) > nt:
                            pop_cmm()
                        pop_mm2(min_q=2)
                # drain the pipeline tails
                while cmm_q:
                    pop_cmm()
                while mm2_q:
                    pop_mm2()

                out_sb = small_pool.tile([1, H], f32, tag="out", name="out")
                for hc in range(HC):
                    nc.vector.tensor_copy(
                        out_sb[:, hc * 512 : (hc + 1) * 512], psC[hc][:]
                    )
                nc.sync.dma_start(acc_d[:], out_sb[:])

            psC = []
            psB_cur = {}
            if reps > 1:
                with tc.For_i(0, reps, 1):
                    emit_body()
            else:
                emit_body()

    nc.compile()
    _compiled_cache[key] = nc
    return nc


def _prep_inputs(input_tensor, Wg, bg, W1, b1, W2, b2):
    """Host-side gating, top-k, gather, bf16 conversion. Returns
    (in_maps, nt, has_b1, csum, total_weight)."""
    B, S, _ = input_tensor.shape
    T = B * S
    x = np.ascontiguousarray(input_tensor.reshape(T, H)).astype(np.float32)

    scores = x @ Wg.astype(np.float32) + bg.astype(np.float32)
    order = np.argsort(-scores, axis=1, kind="stable")
    top_i = order[:, :TOP_K]
    top_v = np.take_along_axis(scores, top_i, axis=1).astype(np.float64)
    ex = np.exp(top_v - top_v.max(axis=1, keepdims=True))
    top_w = ex / ex.sum(axis=1, keepdims=True)
    total_weight = float(top_w.sum())

    flat_e = top_i.ravel()
    flat_t = np.repeat(np.arange(T), TOP_K)
    flat_w = top_w.ravel()
    sort = np.argsort(flat_e, kind="stable")
    flat_e, flat_t, flat_w = flat_e[sort], flat_t[sort], flat_w[sort]
    counts = np.bincount(flat_e, minlength=E)
    starts = np.concatenate([[0], np.cumsum(counts)])

    nt = max(1, math.ceil(counts.max() / 128))
    C = nt * 128

    if USE_FP8:
        # DoubleRow interleave: h = kt*256 + p*2 + j (plain row-major reshape),
        # then fc-major for contiguous per-f-chunk DMAs
        xg = np.zeros((E, KH2, 128, 2, C), dtype=FP8)
        w1_c = (W1.reshape(E, KH2, 128, 2, FC, 512) * W1_SCALE).astype(FP8)
        w1_c = np.ascontiguousarray(
            w1_c.transpose(0, 4, 1, 2, 3, 5)
        ).reshape(E, FC, KH2, 128, 2 * 512)
    else:
        xg = np.zeros((E, H, C), dtype=BF16)
        w1_c = W1.astype(BF16)
    cw = np.zeros((E, C), dtype=BF16)
    csum = np.zeros(E, dtype=np.float64)
    for e in range(E):
        lo, hi = starts[e], starts[e + 1]
        if hi > lo:
            toks = flat_t[lo:hi]
            xt = x[toks].T
            if USE_FP8:
                xg[e, :, :, :, : hi - lo] = xt.astype(FP8).reshape(
                    KH2, 128, 2, hi - lo
                )
            else:
                xg[e, :, : hi - lo] = xt.astype(BF16)
            cw[e, : hi - lo] = flat_w[lo:hi].astype(BF16)
            csum[e] = flat_w[lo:hi].sum()

    w2_bf = W2.astype(BF16)
    has_b1 = bool(np.any(b1))

    in_maps = []
    for i in range(N_CORES):
        m = {
            "xg": xg[EPC * i : EPC * (i + 1)],
            "cw": cw[EPC * i : EPC * (i + 1)],
            "w1": w1_c[EPC * i : EPC * (i + 1)],
            "w2": w2_bf[EPC * i : EPC * (i + 1)],
        }
        if has_b1:
            scale = W1_SCALE if USE_FP8 else 1.0
            m["b1"] = (b1[EPC * i : EPC * (i + 1)] * scale).astype(BF16)
        if not m["xg"].flags["C_CONTIGUOUS"]:
            m = {k: np.ascontiguousarray(v) for k, v in m.items()}
        in_maps.append(m)
    return in_maps, nt, has_b1, csum, total_weight


def _finalize(results, csum, b2, total_weight):
    acc = np.zeros(H, dtype=np.float64)
    for i in range(N_CORES):
        acc += results[i]["acc"].reshape(H).astype(np.float64)
    acc += csum @ b2.astype(np.float64)
    return (acc / total_weight).reshape(1, 1, H).astype(np.float32)


def kernel(input_tensor, Wg, bg, W1, b1, W2, b2):
    in_maps, nt, has_b1, csum, total_weight = _prep_inputs(
        input_tensor, Wg, bg, W1, b1, W2, b2
    )
    nc = _build(nt, has_b1)
    global _last_in_maps
    _last_in_maps = in_maps
    res = run_bass_kernel_spmd(nc, in_maps, core_ids=list(range(N_CORES)))
    return _finalize(res.results, csum, b2, total_weight)


# revision 42
# speedup vs baseline: 1.0771x; 1.0771x over previous
"""Trainium2 Bass kernel for nn_DualOutputMoE.

Math: the reference collapses the whole MoE into a single [1,1,H] vector:
    acc = sum_e combine[:,e] @ (gelu(x @ W1[e] + b1[e]) @ W2[e] + b2[e])
    out = acc / total_weight
Since combine is applied *linearly* after the gelu, we contract it with the
gelu activations first:
    u_e  = combine[:,e] @ gelu(x @ W1[e] + b1[e])          # [F]
    acc  = sum_e (u_e @ W2[e] + combine[:,e].sum() * b2[e])
which turns the second [T,F]@[F,H] matmul into an [F]@[F,H] matvec.
Only tokens actually routed to expert e (combine[:,e] != 0) contribute, so we
gather those tokens on the host (top-2 of 16 experts -> ~T/8 tokens/expert)
and the device computes a dense [C,H]@[H,F] per expert with C = padded
capacity.

Sharding: expert-parallel, 2 experts per core across 8 cores (E=16). Each
core returns its partial [1,H] accumulator; the host sums them.

Device kernel (per core, SPMD over 8 cores). mm1 runs in fp8(e4m3) with
DoubleRow (2 MACs/cell/cycle, measured 2x over bf16); W1 is pre-scaled by 32
into fp8's normal range and the gelu activation un-scales. PSUM accumulation
is fp32 throughout; everything after the gelu is bf16.

Per f-chunk-PAIR (1024 wide, 2 PSUM banks — halves the count of ACT gelu ops,
which are a second bottleneck at ~1.1us each):
  mm1:  psA[tok128, 0:512 | 512:1024] += xgDR[h].T @ W1DR[h, half]  (4 DR k-tiles)
  gelu: G[128,1024] = gelu(psA / 32)           (ACT, psum->sbuf bf16)
  cmm:  psB[1, half] += c[tok128, 1].T @ G[:, half]    (per token-tile, lag-1)
  u     -> [1, F] sbuf row -> DRAM -> uT [128, F/128] (partition transpose)
  mm2:  psC[1, h512] += uT[f128, 1].T @ W2[f128, h512]  (bf16, 32 f-tiles x 2e)
Expert 0's mm2 batches interleave into expert 1's phase 1; W2 tiles prefetch
during phase 1 so mm2 never waits on DMA.
"""

import sys
import math

if "/opt/trn_rl_repo" not in sys.path:
    sys.path.insert(0, "/opt/trn_rl_repo")

import numpy as np
import ml_dtypes

import concourse.bass as bass
import concourse.tile as tile
from concourse import bacc, mybir
from concourse.bass_utils import run_bass_kernel_spmd

BF16 = ml_dtypes.bfloat16
FP8 = ml_dtypes.float8_e4m3
N_CORES = 8
E = 16
EPC = E // N_CORES  # experts per core
H = 1024
F = 4096
TOP_K = 2
KH = H // 128  # 8 bf16 k-tiles along H
KH2 = H // 256  # 4 fp8-DoubleRow k-tiles (256 contraction per tile)
FT = F // 128  # 32 f-tiles along F
FC = F // 512  # 8 f-chunks of 512
FCP = FC // 2  # 4 f-chunk PAIRS of 1024
HC = H // 512  # 2 h-chunks of 512

USE_FP8 = True  # fp8(e4m3) DoubleRow for mm1 (2x PE throughput vs bf16)
W1_SCALE = 32.0  # pre-scale W1 into fp8's normal range; gelu un-scales

_compiled_cache = {}


def _build(nt: int, has_b1: bool, reps: int = 1):
    """Build + compile the SPMD device program for NT token tiles per expert.

    reps > 1 wraps the whole body in a hardware For_i loop running it that
    many times (used by test.py for wall-clock timing).
    """
    key = (nt, has_b1, reps)
    if key in _compiled_cache:
        return _compiled_cache[key]

    C = nt * 128
    nc = bacc.Bacc("TRN2", target_bir_lowering=False, debug=False)
    f32 = mybir.dt.float32
    bf16 = mybir.dt.bfloat16
    fp8 = mybir.dt.float8e4

    NKT = KH2 if USE_FP8 else KH  # mm1 k-tiles
    mm_dt = fp8 if USE_FP8 else bf16
    KW = 2 * 1024 if USE_FP8 else 1024  # w1 free bytes-per-kt per fc-pair (elems)

    if USE_FP8:
        # DoubleRow interleave: h = kt*256 + p*2 + j; fc-pair-major so each
        # pair is one contiguous DMA per k-tile with 2KB lines
        xg_d = nc.dram_tensor(
            "xg", [EPC, KH2, 128, 2, C], fp8, kind="ExternalInput"
        ).ap()
    else:
        xg_d = nc.dram_tensor("xg", [EPC, KH, 128, C], bf16, kind="ExternalInput").ap()
    w1_d = nc.dram_tensor(
        "w1", [EPC, FCP, NKT, 128, KW], mm_dt, kind="ExternalInput"
    ).ap()
    cw_d = nc.dram_tensor("cw", [EPC, C], bf16, kind="ExternalInput").ap()
    w2_d = nc.dram_tensor("w2", [EPC, F, H], bf16, kind="ExternalInput").ap()
    if has_b1:
        b1_d = nc.dram_tensor("b1", [EPC, F], bf16, kind="ExternalInput").ap()
    acc_d = nc.dram_tensor("acc", [1, H], f32, kind="ExternalOutput").ap()
    u_d = nc.dram_tensor("u_scratch", [EPC, F], f32).ap()

    with tile.TileContext(nc) as tc:
        with (
            tc.tile_pool(name="xg", bufs=1) as xg_pool,
            tc.tile_pool(name="cw", bufs=1) as cw_pool,
            tc.tile_pool(name="w1", bufs=2) as w1_pool,
            tc.tile_pool(name="w2", bufs=28) as w2_pool,
            tc.tile_pool(name="g", bufs=8) as g_pool,
            tc.tile_pool(name="u", bufs=1) as u_pool,
            tc.tile_pool(name="small", bufs=1) as small_pool,
            tc.tile_pool(name="psA", bufs=2, space="PSUM") as psA_pool,
            tc.tile_pool(name="psB", bufs=1, space="PSUM") as psB_pool,
            tc.tile_pool(name="psC", bufs=1, space="PSUM") as psC_pool,
        ):
            xg_sb, cw_sb, b1_sb, u8, uT_f, uT_b = [], [], [], [], [], []
            ones_sb = None
            for e in range(EPC):
                if USE_FP8:
                    xg_sb.append(
                        xg_pool.tile(
                            [128, KH2, 2, C], fp8, tag=f"xg{e}", name=f"xg{e}"
                        )
                    )
                else:
                    xg_sb.append(
                        xg_pool.tile(
                            [128, KH, C], bf16, tag=f"xg{e}", name=f"xg{e}"
                        )
                    )
                cw_sb.append(
                    cw_pool.tile([128, nt], bf16, tag=f"cw{e}", name=f"cw{e}")
                )
                u8.append(u_pool.tile([1, F], f32, tag=f"u{e}", name=f"u{e}"))
                uT_f.append(
                    small_pool.tile([128, FT], f32, tag=f"uTf{e}", name=f"uTf{e}")
                )
                uT_b.append(
                    small_pool.tile([128, FT], bf16, tag=f"uTb{e}", name=f"uTb{e}")
                )

            def load_xg(e, kt):
                if USE_FP8:
                    nc.sync.dma_start(xg_sb[e][:, kt, :, :], xg_d[e, kt])
                else:
                    nc.sync.dma_start(xg_sb[e][:, kt, :], xg_d[e, kt])

            def load_cw(e):
                nc.sync.dma_start(
                    cw_sb[e][:], cw_d[e].rearrange("(t p) -> p t", p=128)
                )

            if has_b1:
                ones_sb = small_pool.tile([1, 128], bf16, tag="ones", name="ones")
                nc.vector.memset(ones_sb[:], 1.0)
                for e in range(EPC):
                    b1_t = small_pool.tile([1, F], bf16, tag=f"b1{e}", name=f"b1{e}")
                    nc.sync.dma_start(b1_t[:], b1_d[e : e + 1, :])
                    b1_sb.append(b1_t)

            def w2_prefetch(e, ft):
                w2_t = w2_pool.tile([128, HC, 512], bf16, name="w2t")
                nc.sync.dma_start(
                    w2_t[:],
                    w2_d[e, ft * 128 : (ft + 1) * 128, :].rearrange(
                        "p (h n) -> p h n", h=HC
                    ),
                )
                return w2_t

            def load_w1(e, fp, split=False):
                w1_t = w1_pool.tile([128, NKT, KW], mm_dt, name="w1t")
                if split:  # per-k-tile DMAs: first mm waits on one tile only
                    for kt in range(NKT):
                        nc.sync.dma_start(w1_t[:, kt, :], w1_d[e, fp, kt])
                else:
                    nc.sync.dma_start(
                        w1_t[:], w1_d[e, fp].rearrange("k p n -> p k n")
                    )
                return w1_t

            # software-pipelined emission state
            cmm_q = []  # (e, fp, tt, g_pair)
            mm2_q = []  # (e, [8 w2 tiles], [8 fts])
            psB_cur = {}
            psC = []
            state = {"mm2_count": 0}
            N_MM2 = EPC * FT * HC

            def pop_cmm():
                if not cmm_q:
                    return
                e, fp, tt, g_t = cmm_q.pop(0)
                for half in range(2):
                    nc.tensor.matmul(
                        psB_cur[(e, fp)][:, half * 512 : (half + 1) * 512],
                        lhsT=cw_sb[e][:, tt : tt + 1],
                        rhs=g_t[:, half * 512 : (half + 1) * 512],
                        start=(tt == 0),
                        stop=(tt == nt - 1),
                    )
                if tt == nt - 1:
                    finish_pair(e, fp)

            def finish_pair(e, fp):
                # psB [1,1024] -> u8 cols -> DRAM -> uT columns -> bf16 cast;
                # then queue this pair's mm2 work (8 f-tiles)
                psB = psB_cur.pop((e, fp))
                csl = slice(fp * 1024, (fp + 1) * 1024)
                nc.vector.tensor_copy(u8[e][:, csl], psB[:])
                nc.sync.dma_start(u_d[e : e + 1, csl], u8[e][:, csl])
                nc.sync.dma_start(
                    uT_f[e][:, 8 * fp : 8 * fp + 8],
                    u_d[e, csl].rearrange("(j p) -> p j", p=128),
                )
                nc.vector.tensor_copy(
                    uT_b[e][:, 8 * fp : 8 * fp + 8], uT_f[e][:, 8 * fp : 8 * fp + 8]
                )
                fts = list(range(8 * fp, 8 * fp + 8))
                mm2_q.append((e, [w2_prefetch(e, ft) for ft in fts], fts))

            def pop_mm2(min_q=0):
                if len(mm2_q) <= min_q:
                    return
                e, w2_tiles, fts = mm2_q.pop(0)
                for j, ft in enumerate(fts):
                    for hc in range(HC):
                        # first/last matmul into EACH psC bank get start/stop
                        nc.tensor.matmul(
                            psC[hc][:],
                            lhsT=uT_b[e][:, ft : ft + 1],
                            rhs=w2_tiles[j][:, hc, :],
                            start=(state["mm2_count"] < HC),
                            stop=(state["mm2_count"] >= N_MM2 - HC),
                        )
                        state["mm2_count"] += 1

            def mm1_pair_block(e, fp, w1_t=None):
                """mm1s for one f-chunk pair; one [128,1024] gelu per tt;
                cmms pop with lag-1 so PE never waits on ACT."""
                if w1_t is None:
                    w1_t = load_w1(e, fp)
                psB_cur[(e, fp)] = psB_pool.tile([1, 1024], f32, name="psB")
                for tt in range(nt):
                    psA = psA_pool.tile([128, 1024], f32, name="psA")
                    tsl = slice(tt * 128, (tt + 1) * 128)
                    for kt in range(NKT):
                        for half in range(2):
                            hsl = slice(half * 512, (half + 1) * 512)
                            if USE_FP8:
                                rhs = w1_t[:, kt, :].rearrange(
                                    "p (j n) -> p j n", j=2
                                )[:, :, hsl]
                            else:
                                rhs = w1_t[:, kt, hsl]
                            nc.tensor.matmul(
                                psA[:, hsl],
                                lhsT=(
                                    xg_sb[e][:, kt, :, tsl]
                                    if USE_FP8
                                    else xg_sb[e][:, kt, tsl]
                                ),
                                rhs=rhs,
                                start=(kt == 0),
                                stop=(kt == NKT - 1) and not has_b1,
                                perf_mode=(
                                    mybir.MatmulPerfMode.DoubleRow
                                    if USE_FP8
                                    else None
                                ),
                            )
                    if has_b1:
                        for half in range(2):
                            hsl = slice(half * 512, (half + 1) * 512)
                            nc.tensor.matmul(
                                psA[:, hsl],
                                lhsT=ones_sb[:],
                                rhs=b1_sb[e][
                                    :, fp * 1024 + half * 512 : fp * 1024 + (half + 1) * 512
                                ],
                                start=False,
                                stop=True,
                            )
                    pop_cmm()  # previous tt's cmms (their gelu is long done)
                    g_t = g_pool.tile([128, 1024], bf16, name="gt")
                    nc.scalar.activation(
                        g_t[:],
                        psA[:],
                        mybir.ActivationFunctionType.Gelu,
                        scale=(1.0 / W1_SCALE) if USE_FP8 else 1.0,
                    )
                    cmm_q.append((e, fp, tt, g_t))

            def emit_body():
                psC.clear()
                psC.extend(
                    psC_pool.tile([1, 512], f32, tag=f"psC{hc}", name=f"psC{hc}")
                    for hc in range(HC)
                )
                state["mm2_count"] = 0
                # startup: only e0's k0 slice + w1 pair0 k0 gate the first mm
                load_xg(0, 0)
                load_cw(0)
                w1_first = load_w1(0, 0, split=True)
                for kt in range(1, NKT):
                    load_xg(0, kt)
                for e in range(EPC):
                    for fp in range(FCP):
                        if e == 0 and fp < NKT:  # spread e1's input loads out
                            load_xg(1, fp)
                            if fp == 0:
                                load_cw(1)
                        mm1_pair_block(
                            e, fp, w1_t=w1_first if (e, fp) == (0, 0) else None
                        )
                        pop_mm2(min_q=1)
                while cmm_q:
                    pop_cmm()
                while mm2_q:
                    pop_mm2()

                out_sb = small_pool.tile([1, H], f32, tag="out", name="out")
                for hc in range(HC):
                    nc.vector.tensor_copy(
                        out_sb[:, hc * 512 : (hc + 1) * 512], psC[hc][:]
                    )
                nc.sync.dma_start(acc_d[:], out_sb[:])

            if reps > 1:
                with tc.For_i(0, reps, 1):
                    emit_body()
            else:
                emit_body()

    nc.compile()
    _compiled_cache[key] = nc
    return nc


def _prep_inputs(input_tensor, Wg, bg, W1, b1, W2, b2):
    """Host-side gating, top-k, gather, fp8/bf16 conversion. Returns
    (in_maps, nt, has_b1, csum, total_weight)."""
    B, S, _ = input_tensor.shape
    T = B * S
    x = np.ascontiguousarray(input_tensor.reshape(T, H)).astype(np.float32)

    scores = x @ Wg.astype(np.float32) + bg.astype(np.float32)
    order = np.argsort(-scores, axis=1, kind="stable")
    top_i = order[:, :TOP_K]
    top_v = np.take_along_axis(scores, top_i, axis=1).astype(np.float64)
    ex = np.exp(top_v - top_v.max(axis=1, keepdims=True))
    top_w = ex / ex.sum(axis=1, keepdims=True)
    total_weight = float(top_w.sum())

    flat_e = top_i.ravel()
    flat_t = np.repeat(np.arange(T), TOP_K)
    flat_w = top_w.ravel()
    sort = np.argsort(flat_e, kind="stable")
    flat_e, flat_t, flat_w = flat_e[sort], flat_t[sort], flat_w[sort]
    counts = np.bincount(flat_e, minlength=E)
    starts = np.concatenate([[0], np.cumsum(counts)])

    nt = max(1, math.ceil(counts.max() / 128))
    C = nt * 128

    if USE_FP8:
        # DoubleRow interleave h = kt*256 + p*2 + j, then fc-pair-major
        xg = np.zeros((E, KH2, 128, 2, C), dtype=FP8)
        w1_c = (W1.reshape(E, KH2, 128, 2, FCP, 1024) * W1_SCALE).astype(FP8)
        w1_c = np.ascontiguousarray(w1_c.transpose(0, 4, 1, 2, 3, 5)).reshape(
            E, FCP, KH2, 128, 2 * 1024
        )
    else:
        xg = np.zeros((E, KH, 128, C), dtype=BF16)
        w1_c = W1.reshape(E, KH, 128, FCP, 1024).astype(BF16)
        w1_c = np.ascontiguousarray(w1_c.transpose(0, 3, 1, 2, 4))
    cw = np.zeros((E, C), dtype=BF16)
    csum = np.zeros(E, dtype=np.float64)
    for e in range(E):
        lo, hi = starts[e], starts[e + 1]
        if hi > lo:
            toks = flat_t[lo:hi]
            xt = x[toks].T
            if USE_FP8:
                xg[e, :, :, :, : hi - lo] = xt.astype(FP8).reshape(
                    KH2, 128, 2, hi - lo
                )
            else:
                xg[e, :, :, : hi - lo] = xt.astype(BF16).reshape(KH, 128, hi - lo)
            cw[e, : hi - lo] = flat_w[lo:hi].astype(BF16)
            csum[e] = flat_w[lo:hi].sum()

    w2_bf = W2.astype(BF16)
    has_b1 = bool(np.any(b1))

    in_maps = []
    for i in range(N_CORES):
        m = {
            "xg": np.ascontiguousarray(xg[EPC * i : EPC * (i + 1)]),
            "cw": np.ascontiguousarray(cw[EPC * i : EPC * (i + 1)]),
            "w1": np.ascontiguousarray(w1_c[EPC * i : EPC * (i + 1)]),
            "w2": np.ascontiguousarray(w2_bf[EPC * i : EPC * (i + 1)]),
        }
        if has_b1:
            scale = W1_SCALE if USE_FP8 else 1.0
            m["b1"] = np.ascontiguousarray(
                (b1[EPC * i : EPC * (i + 1)] * scale).astype(BF16)
            )
        in_maps.append(m)
    return in_maps, nt, has_b1, csum, total_weight


def _finalize(results, csum, b2, total_weight):
    acc = np.zeros(H, dtype=np.float64)
    for i in range(N_CORES):
        acc += results[i]["acc"].reshape(H).astype(np.float64)
    acc += csum @ b2.astype(np.float64)
    return (acc / total_weight).reshape(1, 1, H).astype(np.float32)


def kernel(input_tensor, Wg, bg, W1, b1, W2, b2):
    in_maps, nt, has_b1, csum, total_weight = _prep_inputs(
        input_tensor, Wg, bg, W1, b1, W2, b2
    )
    nc = _build(nt, has_b1)
    global _last_in_maps
    _last_in_maps = in_maps
    res = run_bass_kernel_spmd(nc, in_maps, core_ids=list(range(N_CORES)))
    return _finalize(res.results, csum, b2, total_weight)


# revision 49
# speedup vs baseline: 1.1174x; 1.0375x over previous
"""Trainium2 Bass kernel for nn_DualOutputMoE.

Math: the reference collapses the whole MoE into a single [1,1,H] vector:
    acc = sum_e combine[:,e] @ (gelu(x @ W1[e] + b1[e]) @ W2[e] + b2[e])
    out = acc / total_weight
Since combine is applied *linearly* after the gelu, we contract it with the
gelu activations first:
    u_e  = combine[:,e] @ gelu(x @ W1[e] + b1[e])          # [F]
    acc  = sum_e (u_e @ W2[e] + combine[:,e].sum() * b2[e])
which turns the second [T,F]@[F,H] matmul into an [F]@[F,H] matvec.
Only tokens actually routed to expert e (combine[:,e] != 0) contribute, so we
gather those tokens on the host (top-2 of 16 experts -> ~T/8 tokens/expert)
and the device computes a dense [C,H]@[H,F] per expert with C = padded
capacity.

Sharding: expert-parallel, 2 experts per core across 8 cores (E=16). Each
core returns its partial [1,H] accumulator; the host sums them.

Device kernel (per core, SPMD over 8 cores). mm1 runs in fp8(e4m3) with
DoubleRow (2 MACs/cell/cycle, measured 2x over bf16); W1 is pre-scaled by 32
into fp8's normal range and the gelu activation un-scales. PSUM accumulation
is fp32 throughout; everything after the gelu is bf16.

Per f-chunk-PAIR (1024 wide, 2 PSUM banks — halves the count of ACT gelu ops,
which are a second bottleneck at ~1.1us each):
  mm1:  psA[tok128, 0:512 | 512:1024] += xgDR[h].T @ W1DR[h, half]  (4 DR k-tiles)
  gelu: G[128,1024] = gelu(psA / 32)           (ACT, psum->sbuf bf16)
  cmm:  psB[1, half] += c[tok128, 1].T @ G[:, half]    (per token-tile, lag-1)
  u     -> [1, F] sbuf row -> DRAM -> uT [128, F/128] (partition transpose)
  mm2:  psC[1, h512] += uT[f128, 1].T @ W2[f128, h512]  (bf16, 32 f-tiles x 2e)
Expert 0's mm2 batches interleave into expert 1's phase 1; W2 tiles prefetch
during phase 1 so mm2 never waits on DMA.
"""

import sys
import math

if "/opt/trn_rl_repo" not in sys.path:
    sys.path.insert(0, "/opt/trn_rl_repo")

import numpy as np
import ml_dtypes

import concourse.bass as bass
import concourse.tile as tile
from concourse import bacc, mybir
from concourse.bass_utils import run_bass_kernel_spmd

BF16 = ml_dtypes.bfloat16
FP8 = ml_dtypes.float8_e4m3
N_CORES = 8
E = 16
EPC = E // N_CORES  # experts per core
H = 1024
F = 4096
TOP_K = 2
KH = H // 128  # 8 bf16 k-tiles along H
KH2 = H // 256  # 4 fp8-DoubleRow k-tiles (256 contraction per tile)
FT = F // 128  # 32 f-tiles along F
FC = F // 512  # 8 f-chunks of 512
FCP = FC // 2  # 4 f-chunk PAIRS of 1024
HC = H // 512  # 2 h-chunks of 512

USE_FP8 = True  # fp8(e4m3) DoubleRow for mm1 (2x PE throughput vs bf16)
W1_SCALE = 32.0  # pre-scale W1 into fp8's normal range; gelu un-scales

_compiled_cache = {}


def _build(nt: int, has_b1: bool, reps: int = 1):
    """Build + compile the SPMD device program for NT token tiles per expert.

    reps > 1 wraps the whole body in a hardware For_i loop running it that
    many times (used by test.py for wall-clock timing).
    """
    key = (nt, has_b1, reps)
    if key in _compiled_cache:
        return _compiled_cache[key]

    C = nt * 128
    nc = bacc.Bacc("TRN2", target_bir_lowering=False, debug=False)
    f32 = mybir.dt.float32
    bf16 = mybir.dt.bfloat16
    fp8 = mybir.dt.float8e4

    NKT = KH2 if USE_FP8 else KH  # mm1 k-tiles
    mm_dt = fp8 if USE_FP8 else bf16
    KW = 2 * 1024 if USE_FP8 else 1024  # w1 free bytes-per-kt per fc-pair (elems)

    if USE_FP8:
        # DoubleRow interleave: h = kt*256 + p*2 + j; fc-pair-major so each
        # pair is one contiguous DMA per k-tile with 2KB lines
        xg_d = nc.dram_tensor(
            "xg", [EPC, KH2, 128, 2, C], fp8, kind="ExternalInput"
        ).ap()
    else:
        xg_d = nc.dram_tensor("xg", [EPC, KH, 128, C], bf16, kind="ExternalInput").ap()
    w1_d = nc.dram_tensor(
        "w1", [EPC, FCP, NKT, 128, KW], mm_dt, kind="ExternalInput"
    ).ap()
    cw_d = nc.dram_tensor("cw", [EPC, C], bf16, kind="ExternalInput").ap()
    w2_d = nc.dram_tensor("w2", [EPC, F, H], bf16, kind="ExternalInput").ap()
    if has_b1:
        b1_d = nc.dram_tensor("b1", [EPC, F], bf16, kind="ExternalInput").ap()
    acc_d = nc.dram_tensor("acc", [1, H], f32, kind="ExternalOutput").ap()
    u_d = nc.dram_tensor("u_scratch", [EPC, F], f32).ap()

    with tile.TileContext(nc) as tc:
        with (
            tc.tile_pool(name="xg", bufs=1) as xg_pool,
            tc.tile_pool(name="cw", bufs=1) as cw_pool,
            tc.tile_pool(name="w1", bufs=2) as w1_pool,
            tc.tile_pool(name="w2", bufs=28) as w2_pool,
            tc.tile_pool(name="g", bufs=8) as g_pool,
            tc.tile_pool(name="u", bufs=1) as u_pool,
            tc.tile_pool(name="small", bufs=1) as small_pool,
            tc.tile_pool(name="psA", bufs=2, space="PSUM") as psA_pool,
            tc.tile_pool(name="psB", bufs=1, space="PSUM") as psB_pool,
            tc.tile_pool(name="psC", bufs=1, space="PSUM") as psC_pool,
        ):
            xg_sb, cw_sb, b1_sb, u8, uT_f, uT_b = [], [], [], [], [], []
            ones_sb = None
            for e in range(EPC):
                if USE_FP8:
                    xg_sb.append(
                        xg_pool.tile(
                            [128, KH2, 2, C], fp8, tag=f"xg{e}", name=f"xg{e}"
                        )
                    )
                else:
                    xg_sb.append(
                        xg_pool.tile(
                            [128, KH, C], bf16, tag=f"xg{e}", name=f"xg{e}"
                        )
                    )
                cw_sb.append(
                    cw_pool.tile([128, nt], bf16, tag=f"cw{e}", name=f"cw{e}")
                )
                u8.append(u_pool.tile([1, F], f32, tag=f"u{e}", name=f"u{e}"))
                uT_f.append(
                    small_pool.tile([128, FT], f32, tag=f"uTf{e}", name=f"uTf{e}")
                )
                uT_b.append(
                    small_pool.tile([128, FT], bf16, tag=f"uTb{e}", name=f"uTb{e}")
                )

            def load_xg(e, kt):
                if USE_FP8:
                    nc.sync.dma_start(xg_sb[e][:, kt, :, :], xg_d[e, kt])
                else:
                    nc.sync.dma_start(xg_sb[e][:, kt, :], xg_d[e, kt])

            def load_cw(e):
                nc.sync.dma_start(
                    cw_sb[e][:], cw_d[e].rearrange("(t p) -> p t", p=128)
                )

            if has_b1:
                ones_sb = small_pool.tile([1, 128], bf16, tag="ones", name="ones")
                nc.vector.memset(ones_sb[:], 1.0)
                for e in range(EPC):
                    b1_t = small_pool.tile([1, F], bf16, tag=f"b1{e}", name=f"b1{e}")
                    nc.sync.dma_start(b1_t[:], b1_d[e : e + 1, :])
                    b1_sb.append(b1_t)

            def w2_prefetch(e, ft):
                w2_t = w2_pool.tile([128, HC, 512], bf16, name="w2t")
                nc.sync.dma_start(
                    w2_t[:],
                    w2_d[e, ft * 128 : (ft + 1) * 128, :].rearrange(
                        "p (h n) -> p h n", h=HC
                    ),
                )
                return w2_t

            def load_w1(e, fp, split=False):
                w1_t = w1_pool.tile([128, NKT, KW], mm_dt, name="w1t")
                if split:  # per-k-tile DMAs: first mm waits on one tile only
                    for kt in range(NKT):
                        nc.sync.dma_start(w1_t[:, kt, :], w1_d[e, fp, kt])
                else:
                    nc.sync.dma_start(
                        w1_t[:], w1_d[e, fp].rearrange("k p n -> p k n")
                    )
                return w1_t

            # software-pipelined emission state
            cmm_q = []  # (e, fp, tt, g_pair)
            mm2_q = []  # (e, [8 w2 tiles], [8 fts])
            psB_cur = {}
            psC = []
            state = {"mm2_count": 0}
            N_MM2 = EPC * FT * HC

            def pop_cmm():
                if not cmm_q:
                    return
                e, fp, tt, g_t = cmm_q.pop(0)
                for half in range(2):
                    nc.tensor.matmul(
                        psB_cur[(e, fp)][:, half * 512 : (half + 1) * 512],
                        lhsT=cw_sb[e][:, tt : tt + 1],
                        rhs=g_t[:, half * 512 : (half + 1) * 512],
                        start=(tt == 0),
                        stop=(tt == nt - 1),
                    )
                if tt == nt - 1:
                    finish_pair(e, fp)

            def finish_pair(e, fp):
                # psB [1,1024] -> u8 cols -> DRAM -> uT columns -> bf16 cast;
                # then queue this pair's mm2 work (8 f-tiles)
                psB = psB_cur.pop((e, fp))
                csl = slice(fp * 1024, (fp + 1) * 1024)
                nc.vector.tensor_copy(u8[e][:, csl], psB[:])
                nc.sync.dma_start(u_d[e : e + 1, csl], u8[e][:, csl])
                nc.sync.dma_start(
                    uT_f[e][:, 8 * fp : 8 * fp + 8],
                    u_d[e, csl].rearrange("(j p) -> p j", p=128),
                )
                nc.vector.tensor_copy(
                    uT_b[e][:, 8 * fp : 8 * fp + 8], uT_f[e][:, 8 * fp : 8 * fp + 8]
                )
                fts = list(range(8 * fp, 8 * fp + 8))
                mm2_q.append((e, [w2_prefetch(e, ft) for ft in fts], fts))

            def pop_mm2(min_q=0):
                if len(mm2_q) <= min_q:
                    return
                e, w2_tiles, fts = mm2_q.pop(0)
                for j, ft in enumerate(fts):
                    for hc in range(HC):
                        # first/last matmul into EACH psC bank get start/stop
                        nc.tensor.matmul(
                            psC[hc][:],
                            lhsT=uT_b[e][:, ft : ft + 1],
                            rhs=w2_tiles[j][:, hc, :],
                            start=(state["mm2_count"] < HC),
                            stop=(state["mm2_count"] >= N_MM2 - HC),
                        )
                        state["mm2_count"] += 1

            def mm1_pair_block(e, fp, w1_t=None):
                """mm1s for one f-chunk pair; one [128,1024] gelu per tt;
                cmms pop with lag-1 so PE never waits on ACT."""
                if w1_t is None:
                    w1_t = load_w1(e, fp)
                psB_cur[(e, fp)] = psB_pool.tile([1, 1024], f32, name="psB")
                for tt in range(nt):
                    psA = psA_pool.tile([128, 1024], f32, name="psA")
                    tsl = slice(tt * 128, (tt + 1) * 128)
                    for kt in range(NKT):
                        for half in range(2):
                            hsl = slice(half * 512, (half + 1) * 512)
                            if USE_FP8:
                                rhs = w1_t[:, kt, :].rearrange(
                                    "p (j n) -> p j n", j=2
                                )[:, :, hsl]
                            else:
                                rhs = w1_t[:, kt, hsl]
                            nc.tensor.matmul(
                                psA[:, hsl],
                                lhsT=(
                                    xg_sb[e][:, kt, :, tsl]
                                    if USE_FP8
                                    else xg_sb[e][:, kt, tsl]
                                ),
                                rhs=rhs,
                                start=(kt == 0),
                                stop=(kt == NKT - 1) and not has_b1,
                                perf_mode=(
                                    mybir.MatmulPerfMode.DoubleRow
                                    if USE_FP8
                                    else None
                                ),
                            )
                    if has_b1:
                        for half in range(2):
                            hsl = slice(half * 512, (half + 1) * 512)
                            nc.tensor.matmul(
                                psA[:, hsl],
                                lhsT=ones_sb[:],
                                rhs=b1_sb[e][
                                    :, fp * 1024 + half * 512 : fp * 1024 + (half + 1) * 512
                                ],
                                start=False,
                                stop=True,
                            )
                    pop_cmm()  # previous tt's cmms (their gelu is long done)
                    g_t = g_pool.tile([128, 1024], bf16, name="gt")
                    nc.scalar.activation(
                        g_t[:],
                        psA[:],
                        mybir.ActivationFunctionType.Gelu,
                        scale=(1.0 / W1_SCALE) if USE_FP8 else 1.0,
                    )
                    cmm_q.append((e, fp, tt, g_t))

            def emit_body():
                psC.clear()
                psC.extend(
                    psC_pool.tile([1, 512], f32, tag=f"psC{hc}", name=f"psC{hc}")
                    for hc in range(HC)
                )
                state["mm2_count"] = 0
                # startup: only e0's k0 slice + w1 pair0 k0 gate the first mm
                load_xg(0, 0)
                load_cw(0)
                w1_first = load_w1(0, 0, split=True)
                for kt in range(1, NKT):
                    load_xg(0, kt)
                for e in range(EPC):
                    for fp in range(FCP):
                        if e == 0 and fp < NKT:  # spread e1's input loads out
                            load_xg(1, fp)
                            if fp == 0:
                                load_cw(1)
                        mm1_pair_block(
                            e, fp, w1_t=w1_first if (e, fp) == (0, 0) else None
                        )
                        pop_mm2(min_q=1)
                while cmm_q:
                    pop_cmm()
                while mm2_q:
                    pop_mm2()

                out_sb = small_pool.tile([1, H], f32, tag="out", name="out")
                for hc in range(HC):
                    nc.vector.tensor_copy(
                        out_sb[:, hc * 512 : (hc + 1) * 512], psC[hc][:]
                    )
                nc.sync.dma_start(acc_d[:], out_sb[:])

            if reps > 1:
                with tc.For_i(0, reps, 1):
                    emit_body()
            else:
                emit_body()

    nc.compile()
    _compiled_cache[key] = nc
    return nc


def _prep_inputs(input_tensor, Wg, bg, W1, b1, W2, b2):
    """Host-side gating, top-k, gather, fp8/bf16 conversion. Returns
    (in_maps, nt, has_b1, csum, total_weight)."""
    B, S, _ = input_tensor.shape
    T = B * S
    x = np.ascontiguousarray(input_tensor.reshape(T, H)).astype(np.float32)

    scores = x @ Wg.astype(np.float32) + bg.astype(np.float32)
    order = np.argsort(-scores, axis=1, kind="stable")
    top_i = order[:, :TOP_K]
    top_v = np.take_along_axis(scores, top_i, axis=1).astype(np.float64)
    ex = np.exp(top_v - top_v.max(axis=1, keepdims=True))
    top_w = ex / ex.sum(axis=1, keepdims=True)
    total_weight = float(top_w.sum())

    flat_e = top_i.ravel()
    flat_t = np.repeat(np.arange(T), TOP_K)
    flat_w = top_w.ravel()
    sort = np.argsort(flat_e, kind="stable")
    flat_e, flat_t, flat_w = flat_e[sort], flat_t[sort], flat_w[sort]
    counts = np.bincount(flat_e, minlength=E)
    starts = np.concatenate([[0], np.cumsum(counts)])

    nt = max(1, math.ceil(counts.max() / 128))
    C = nt * 128

    if USE_FP8:
        # DoubleRow interleave h = kt*256 + p*2 + j, then fc-pair-major
        xg = np.zeros((E, KH2, 128, 2, C), dtype=FP8)
        w1_c = (W1.reshape(E, KH2, 128, 2, FCP, 1024) * W1_SCALE).astype(FP8)
        w1_c = np.ascontiguousarray(w1_c.transpose(0, 4, 1, 2, 3, 5)).reshape(
            E, FCP, KH2, 128, 2 * 1024
        )
    else:
        xg = np.zeros((E, KH, 128, C), dtype=BF16)
        w1_c = W1.reshape(E, KH, 128, FCP, 1024).astype(BF16)
        w1_c = np.ascontiguousarray(w1_c.transpose(0, 3, 1, 2, 4))
    cw = np.zeros((E, C), dtype=BF16)
    csum = np.zeros(E, dtype=np.float64)
    for e in range(E):
        lo, hi = starts[e], starts[e + 1]
        if hi > lo:
            toks = flat_t[lo:hi]
            xt = x[toks].T
            if USE_FP8:
                xg[e, :, :, :, : hi - lo] = xt.astype(FP8).reshape(
                    KH2, 128, 2, hi - lo
                )
            else:
                xg[e, :, :, : hi - lo] = xt.astype(BF16).reshape(KH, 128, hi - lo)
            cw[e, : hi - lo] = flat_w[lo:hi].astype(BF16)
            csum[e] = flat_w[lo:hi].sum()

    w2_bf = W2.astype(BF16)
    has_b1 = bool(np.any(b1))

    in_maps = []
    for i in range(N_CORES):
        m = {
            "xg": np.ascontiguousarray(xg[EPC * i : EPC * (i + 1)]),
            "cw": np.ascontiguousarray(cw[EPC * i : EPC * (i + 1)]),
            "w1": np.ascontiguousarray(w1_c[EPC * i : EPC * (i + 1)]),
            "w2": np.ascontiguousarray(w2_bf[EPC * i : EPC * (i + 1)]),
        }
        if has_b1:
            scale = W1_SCALE if USE_FP8 else 1.0
            m["b1"] = np.ascontiguousarray(
                (b1[EPC * i : EPC * (i + 1)] * scale).astype(BF16)
            )
        in_maps.append(m)
    return in_maps, nt, has_b1, csum, total_weight


def _finalize(results, csum, b2, total_weight):
    acc = np.zeros(H, dtype=np.float64)
    for i in range(N_CORES):
        acc += results[i]["acc"].reshape(H).astype(np.float64)
    acc += csum @ b2.astype(np.float64)
    return (acc / total_weight).reshape(1, 1, H).astype(np.float32)


def kernel(input_tensor, Wg, bg, W1, b1, W2, b2):
    in_maps, nt, has_b1, csum, total_weight = _prep_inputs(
        input_tensor, Wg, bg, W1, b1, W2, b2
    )
    nc = _build(nt, has_b1)
    global _last_in_maps
    _last_in_maps = in_maps
    res = run_bass_kernel_spmd(nc, in_maps, core_ids=list(range(N_CORES)))
    return _finalize(res.results, csum, b2, total_weight)
